# revision 40
# baseline (speedup 1.0000x reference)
"""Trainium2 Bass kernel for GQA attention (B=2, S=2048, DIM=2048, H=32, KV=8, HD=64).

Sharding: tensor-parallel over kv heads (TP=4, 2 kv heads / 8 q heads per core)
x data-parallel over batch (DP=2).  Core c = d*4 + t.  Each core computes a
partial out = attn_out_shard @ wo_rows_shard for its batch; the host sums the
4 TP partials per batch.

All host-side work is layout-only: transpose x, permute wq/wk columns into a
RoPE-friendly even/odd layout, cast to bf16, build trig/mask pattern tiles.

Device dataflow (per core):
 - projections with x^T resident in SBUF (bf16 matmuls, k-outer accumulation)
 - RoPE via stream_shuffle + two tensor muls + add (even/odd pairs laid out
   within 32-partition quadrants)
 - attention with transposed scores (scores[sk, sq]) so no transposes are
   needed anywhere in the inner loop; exp on ScalarE with no max-subtraction
   (inputs are unit-scale; softmax is shift-invariant)
 - causal masking by block skipping + multiplicative 0/1 patterns on the
   diagonal tiles, with column-trimmed exp/mask/av on those tiles
 - softmax denominators ride as a ones-column inside the AV matmul lhsT;
   normalization = DVE reciprocal -> gpsimd partition_broadcast -> DVE mul
 - wo output projection as a final phase, outputs streamed to DRAM
"""

import os
import sys

import numpy as np

_REPO = "/opt/trn_rl_repo"
if _REPO not in sys.path:
    sys.path.insert(0, _REPO)

import ml_dtypes  # noqa: E402

BF16 = ml_dtypes.bfloat16

B, S, DIM = 2, 2048, 2048
H, KV, HD = 32, 8, 64
TP, DP = 4, 2
NCORES = TP * DP
HQ = (H // TP) * HD          # 512 q-proj cols per core
HKV = (KV // TP) * HD        # 128 kv-proj cols per core
NKVC = KV // TP              # 2 kv heads per core
NPAIR = (H // TP) // 2       # 4 q-head pairs per core
SQC = 512                    # sq chunk width
NCHUNK = S // SQC
SKT = 128                    # sk tile height
NSKT = S // SKT
KT = DIM // 128              # contraction tiles
VW = 130                     # v_sb tile: [0(32) | 1 | 0(31) | v(64) | 1 | pad]

# RoPE layout: within each head's 64 dims -> 64 partitions, quadrant q (32)
# holds pairs 16q..16q+15 as [evens(16) | odds(16)].
_perm = np.empty(64, np.int64)
_freq = np.empty(64, np.int64)
_sgn = np.empty(64, np.float32)
for _p in range(64):
    _q, _j = divmod(_p, 32)
    if _j < 16:
        _i = 16 * _q + _j
        _perm[_p] = 2 * _i
        _sgn[_p] = -1.0
    else:
        _i = 16 * _q + _j - 16
        _perm[_p] = 2 * _i + 1
        _sgn[_p] = 1.0
    _freq[_p] = _i
SHUF = list(range(16, 32)) + list(range(0, 16))

_build_cache = {}
last_exec_time_ns = None
last_trace = None


def _mask_structure(mask):
    """chunks[c] = [(t, pat_idx|None, col_trim), ...] per valid sk tile;
    patterns = list of [128, 2*SQC] float32 0/1 (duplicated for both halves
    of the mixed-half p tile)."""
    valid = mask[0, 0] == 0.0  # [sq, sk]
    chunks = []
    patterns = []
    pat_keys = {}
    for c in range(NCHUNK):
        glist = []
        for t in range(NSKT):
            sub = valid[c * SQC:(c + 1) * SQC, t * SKT:(t + 1) * SKT]
            if not sub.any():
                continue
            if sub.all():
                glist.append((t, None, 0))
                continue
            pat = np.empty((128, 2 * SQC), np.float32)
            pat[:, 0:SQC] = sub.T
            pat[:, SQC:2 * SQC] = sub.T
            key = pat.tobytes()
            if key not in pat_keys:
                pat_keys[key] = len(patterns)
                patterns.append(pat)
            # first sq column with any valid element: exp/mask/av can skip
            # columns < r (their p values are zero / never contribute)
            r = int(np.argmax(sub.any(axis=1)))
            glist.append((t, pat_keys[key], r))
        chunks.append(glist)
    return chunks, patterns


def _build(chunks, n_pat):
    import concourse.bass as bass  # noqa: F401
    import concourse.mybir as mybir
    from concourse import bacc
    from concourse.masks import make_identity
    from concourse.tile import TileContext

    F32, BF = mybir.dt.float32, mybir.dt.bfloat16
    MUL = mybir.AluOpType.mult
    ADD = mybir.AluOpType.add
    EXP = mybir.ActivationFunctionType.Exp

    nc = bacc.Bacc()
    xt_e = nc.declare_dram_parameter("xt", [DIM, S], BF, isOutput=False)
    wq_e = nc.declare_dram_parameter("wq", [DIM, HQ], BF, isOutput=False)
    wk_e = nc.declare_dram_parameter("wk", [DIM, HKV], BF, isOutput=False)
    wv_e = nc.declare_dram_parameter("wv", [DIM, HKV], BF, isOutput=False)
    wo_e = nc.declare_dram_parameter("wo", [HQ, DIM], BF, isOutput=False)
    c1_e = nc.declare_dram_parameter("c1", [128, S], BF, isOutput=False)
    c2_e = nc.declare_dram_parameter("c2", [128, S], BF, isOutput=False)
    dm_e = nc.declare_dram_parameter("dmask", [128, n_pat * 2 * SQC], BF,
                                     isOutput=False)
    out_e = nc.declare_dram_parameter("out", [S, DIM], BF, isOutput=True)

    with TileContext(nc) as tc:
        with tc.tile_pool(name="persist", bufs=1) as P:
            q_t = [P.tile([128, S], BF, tag=f"q{j}", name=f"q{j}")
                   for j in range(NPAIR)]
            k_t = P.tile([128, S], BF, tag="kt")
            v_sb = [P.tile([128, NSKT * VW], BF, tag=f"v{g}", name=f"v{g}")
                    for g in range(NKVC)]
            attn = [P.tile([128, S], BF, tag=f"a{j}", name=f"a{j}")
                    for j in range(NPAIR)]
            wo_sb = [P.tile([128, DIM], BF, tag=f"wo{j}", name=f"wo{j}")
                     for j in range(NPAIR)]
            dm_sb = P.tile([128, n_pat * 2 * SQC], BF, tag="dm")

            for j in range(NPAIR):
                nc.gpsimd.dma_start(out=wo_sb[j],
                                    in_=wo_e[128 * j:128 * (j + 1), :])
            nc.gpsimd.dma_start(out=dm_sb, in_=dm_e[:, :])

            # v background: [0(32) | 1 | 0(31) | v | 1 | pad] per sk tile
            for g in range(NKVC):
                v3 = v_sb[g].rearrange("p (t w) -> p t w", w=VW)
                nc.vector.memset(v3[:, :, 0:32], 0.0)
                nc.vector.memset(v3[:, :, 32:33], 1.0)
                nc.vector.memset(v3[:, :, 33:64], 0.0)
                nc.vector.memset(v3[:, :, 128:129], 1.0)

            # ---------------- projections ----------------
            with (
                tc.tile_pool(name="xw", bufs=1) as XW,
                tc.tile_pool(name="ropew", bufs=2) as W,
                tc.tile_pool(name="pps", bufs=2, space="PSUM") as PPS,
            ):
                c1_sb = XW.tile([128, S], BF, tag="c1")
                c2_sb = XW.tile([128, S], BF, tag="c2")
                nc.gpsimd.dma_start(out=c1_sb, in_=c1_e[:, :])
                nc.gpsimd.dma_start(out=c2_sb, in_=c2_e[:, :])
                # x on the sync HWDGE queue, weights on the scalar queue, so
                # weight tiles land in parallel with the big x stream
                xt_sb, wq_sb, wk_sb, wv_sb = [], [], [], []
                for k in range(KT):
                    xk = XW.tile([128, S], BF, tag=f"x{k}")
                    nc.sync.dma_start(out=xk,
                                      in_=xt_e[128 * k:128 * (k + 1), :])
                    xt_sb.append(xk)
                    kk = XW.tile([128, HKV], BF, tag=f"wk{k}")
                    nc.scalar.dma_start(out=kk,
                                        in_=wk_e[128 * k:128 * (k + 1), :])
                    wk_sb.append(kk)
                    vk = XW.tile([128, HKV], BF, tag=f"wv{k}")
                    nc.scalar.dma_start(out=vk,
                                        in_=wv_e[128 * k:128 * (k + 1), :])
                    wv_sb.append(vk)
                    qk_ = XW.tile([128, HQ], BF, tag=f"wq{k}")
                    nc.scalar.dma_start(out=qk_,
                                        in_=wq_e[128 * k:128 * (k + 1), :])
                    wq_sb.append(qk_)

                def rope(dst, raw):
                    # dst = raw*c1 + shuffle(raw)*c2
                    sh = W.tile([128, S], BF, tag="sh", name="sh")
                    t1 = W.tile([128, S], BF, tag="t1", name="t1")
                    nc.vector.stream_shuffle(sh, raw, SHUF)
                    nc.vector.tensor_tensor(t1, raw, c1_sb, MUL)
                    nc.vector.tensor_tensor(sh, sh, c2_sb, MUL)
                    nc.vector.tensor_tensor(dst, t1, sh, ADD)

                def rope_project(dst, w_tiles, col0):
                    # k-outer: one ldweights feeds 4 chunk matmuls
                    raw = W.tile([128, S], BF, tag="qraw", name="raw")
                    _tags = ["ppk0", "ppk1", "ppv0", "ppv1"]
                    pss = [PPS.tile([128, SQC], F32, tag=_tags[c],
                                    name=f"pq{c}", bufs=1)
                           for c in range(NCHUNK)]
                    for k in range(KT):
                        for c in range(NCHUNK):
                            nc.tensor.matmul(
                                pss[c],
                                w_tiles[k][:, col0:col0 + 128],
                                xt_sb[k][:, SQC * c:SQC * (c + 1)],
                                start=(k == 0), stop=(k == KT - 1),
                            )
                    for c in range(NCHUNK):
                        nc.scalar.copy(raw[:, SQC * c:SQC * (c + 1)], pss[c])
                    rope(dst, raw)

                # k / v_t / q0 interleaved per k-tile in two chunk passes so
                # the PE has dense work while x/w tiles stream in
                ident = XW.tile([128, 128], BF, tag="ident")
                make_identity(nc, ident)
                raw_k = W.tile([128, S], BF, tag="rawk", bufs=1)
                raw_q0 = W.tile([128, S], BF, tag="rawq0", bufs=1)
                raw_q1 = W.tile([128, S], BF, tag="rawq1", bufs=1)
                vt_raw = W.tile([128, S], BF, tag="rawv", bufs=1)
                for crng in ((0, 1), (2, 3)):
                    ps_k = [PPS.tile([128, SQC], F32, tag=f"ppk{i}",
                                     name=f"ppk{i}", bufs=1)
                            for i in range(2)]
                    ps_v = [PPS.tile([128, SQC], F32, tag=f"ppv{i}",
                                     name=f"ppv{i}", bufs=1)
                            for i in range(2)]
                    ps_q = [PPS.tile([128, SQC], F32, tag=f"ppq{i}",
                                     name=f"ppq{i}", bufs=1)
                            for i in range(2)]
                    ps_q1 = [PPS.tile([128, SQC], F32, tag=f"ppr{i}",
                                      name=f"ppr{i}", bufs=1)
                             for i in range(2)]
                    for k in range(KT):
                        for ci, c in enumerate(crng):
                            xs = xt_sb[k][:, SQC * c:SQC * (c + 1)]
                            nc.tensor.matmul(
                                ps_k[ci], wk_sb[k], xs,
                                start=(k == 0), stop=(k == KT - 1))
                            nc.tensor.matmul(
                                ps_v[ci], wv_sb[k], xs,
                                start=(k == 0), stop=(k == KT - 1))
                            nc.tensor.matmul(
                                ps_q[ci], wq_sb[k][:, 0:128], xs,
                                start=(k == 0), stop=(k == KT - 1))
                            nc.tensor.matmul(
                                ps_q1[ci], wq_sb[k][:, 128:256], xs,
                                start=(k == 0), stop=(k == KT - 1))
                    for ci, c in enumerate(crng):
                        sl = slice(SQC * c, SQC * (c + 1))
                        nc.scalar.copy(raw_k[:, sl], ps_k[ci])
                        nc.scalar.copy(vt_raw[:, sl], ps_v[ci])
                        nc.scalar.copy(raw_q0[:, sl], ps_q[ci])
                        nc.scalar.copy(raw_q1[:, sl], ps_q1[ci])
                rope(k_t, raw_k)
                rope(q_t[0], raw_q0)
                rope(q_t[1], raw_q1)
                for t in range(NSKT):
                    tp = PPS.tile([128, 128], BF, tag="ppk0", bufs=1)
                    nc.tensor.transpose(tp, vt_raw[:, SKT * t:SKT * (t + 1)],
                                        ident)
                    nc.scalar.copy(
                        v_sb[0][:, VW * t + 64:VW * t + 128], tp[:, 0:64])
                    nc.scalar.copy(
                        v_sb[1][:, VW * t + 64:VW * t + 128], tp[:, 64:128])

                for j in range(2, NPAIR):
                    rope_project(q_t[j], wq_sb, 128 * j)

            # ---------------- attention ----------------
            with (
                tc.tile_pool(name="attw", bufs=2) as W,
                tc.tile_pool(name="scps", bufs=2, space="PSUM") as SCPS,
                tc.tile_pool(name="avps", bufs=1, space="PSUM") as AVPS,
            ):
                for c in range(NCHUNK):
                    glist = chunks[c]
                    for j in range(NPAIR):
                        # pair j = (q-head j -> kv 0, q-head j+4 -> kv 1)
                        # mixed-half sc tile: lo head at cols 0:512 (PE rows
                        # 0-63), hi head at cols 512:1024 (rows 64-127) --
                        # the two qk matmuls run concurrently in the array
                        pp = j % 2
                        av_lo = AVPS.tile([128, SQC], F32, tag=f"avlo{pp}",
                                          name="av_lo")
                        av_hi = AVPS.tile([128, SQC], F32, tag=f"avhi{pp}",
                                          name="av_hi")
                        for ti, (t, patk, r) in enumerate(glist):
                            first = ti == 0
                            last = ti == len(glist) - 1
                            if first:
                                r = 0  # first av matmul must cover all cols
                            sc = SCPS.tile([128, 2 * SQC], F32,
                                           tag="sc", name="sc")
                            nc.tensor.matmul(
                                sc[:, r:SQC],
                                k_t[0:64, SKT * t:SKT * (t + 1)],
                                q_t[j][0:64, SQC * c + r:SQC * (c + 1)],
                                start=True, stop=True,
                            )
                            nc.tensor.matmul(
                                sc[:, SQC + r:2 * SQC],
                                k_t[64:128, SKT * t:SKT * (t + 1)],
                                q_t[j][64:128, SQC * c + r:SQC * (c + 1)],
                                start=True, stop=True,
                            )
                            p = W.tile([128, 2 * SQC], BF, tag="p", name="p",
                                       bufs=4)
                            if r:
                                sc3 = sc.rearrange(
                                    "q (h f) -> q h f", h=2)[:, :, r:SQC]
                                p3 = p.rearrange(
                                    "q (h f) -> q h f", h=2)[:, :, r:SQC]
                                nc.scalar.activation(p3, sc3, EXP,
                                                     scale=0.125)
                            else:
                                nc.scalar.activation(p, sc, EXP, scale=0.125)
                            if patk is not None:
                                dslice = dm_sb[:, 2 * SQC * patk:
                                               2 * SQC * (patk + 1)]
                                if r:
                                    d3 = dslice.rearrange(
                                        "q (h f) -> q h f", h=2)[:, :, r:SQC]
                                    nc.vector.tensor_tensor(p3, p3, d3, MUL)
                                else:
                                    nc.vector.tensor_tensor(p, p, dslice, MUL)
                            nc.tensor.matmul(
                                av_lo[0:65, r:SQC],
                                v_sb[0][:, VW * t + 64:VW * t + 129],
                                p[:, r:SQC],
                                start=first, stop=last,
                            )
                            nc.tensor.matmul(
                                av_hi[0:128, r:SQC],
                                v_sb[1][:, VW * t:VW * t + 128],
                                p[:, SQC + r:2 * SQC],
                                start=first, stop=last,
                            )
                        rec_lo = W.tile([1, SQC], F32, tag="reclo",
                                        name="rec_lo")
                        rec_hi = W.tile([1, SQC], F32, tag="rechi",
                                        name="rec_hi")
                        rb_lo = W.tile([128, SQC], F32, tag="rblo",
                                       name="rb_lo")
                        rb_hi = W.tile([128, SQC], F32, tag="rbhi",
                                       name="rb_hi")
                        nc.vector.reciprocal(rec_lo[0:1, :], av_lo[64:65, :])
                        nc.gpsimd.partition_broadcast(rb_lo, rec_lo[0:1, :])
                        nc.vector.tensor_tensor(
                            attn[j][0:64, SQC * c:SQC * (c + 1)],
                            av_lo[0:64, :], rb_lo[0:64, :], MUL)
                        nc.vector.reciprocal(rec_hi[0:1, :], av_hi[32:33, :])
                        nc.gpsimd.partition_broadcast(rb_hi, rec_hi[0:1, :])
                        nc.vector.tensor_tensor(
                            attn[j][64:128, SQC * c:SQC * (c + 1)],
                            av_hi[64:128, :], rb_hi[64:128, :], MUL)

            # ---------------- wo (output projection) ----------------
            with (
                tc.tile_pool(name="wow", bufs=3) as W2,
                tc.tile_pool(name="ops", bufs=4, space="PSUM") as OPS,
            ):
                for s in range(S // 128):
                    o_sb = W2.tile([128, DIM], BF, tag="osb", name="o_sb")
                    for n in range(DIM // 512):
                        pso = OPS.tile([128, 512], F32, tag="pso",
                                       name="pso")
                        for j in range(NPAIR):
                            nc.tensor.matmul(
                                pso,
                                attn[j][:, 128 * s:128 * (s + 1)],
                                wo_sb[j][:, 512 * n:512 * (n + 1)],
                                start=(j == 0), stop=(j == NPAIR - 1),
                            )
                        dst = o_sb[:, 512 * n:512 * (n + 1)]
                        if n % 2 == 0:
                            nc.vector.tensor_copy(dst, pso)
                        else:
                            nc.scalar.copy(dst, pso)
                    nc.sync.dma_start(out=out_e[128 * s:128 * (s + 1), :],
                                      in_=o_sb)

    nc.finalize()
    return nc


def kernel(**inputs):
    global last_exec_time_ns, last_trace
    from concourse.bass_utils import run_bass_kernel_spmd

    x = np.asarray(inputs["x"], np.float32)
    freqs_cos = np.asarray(inputs["freqs_cos"], np.float32)
    freqs_sin = np.asarray(inputs["freqs_sin"], np.float32)
    mask = np.asarray(inputs["mask"], np.float32)
    wq = np.asarray(inputs["wq"], np.float32)
    wk = np.asarray(inputs["wk"], np.float32)
    wv = np.asarray(inputs["wv"], np.float32)
    wo = np.asarray(inputs["wo"], np.float32)

    chunks, patterns = _mask_structure(mask)
    n_pat = max(len(patterns), 1)
    if patterns:
        dmask = np.concatenate(patterns, axis=1).astype(BF16)
    else:
        dmask = np.ones((128, 2 * SQC), np.float32).astype(BF16)

    key = tuple(tuple(g) for g in chunks)
    if key not in _build_cache:
        _build_cache[key] = _build(chunks, n_pat)
    nc = _build_cache[key]

    # trig tiles in pair layout (same for both heads of a pair)
    fi2 = np.tile(_freq, 2)
    sg2 = np.tile(_sgn, 2)
    c1 = freqs_cos.T[fi2].astype(BF16)                      # [128, S]
    c2 = (freqs_sin.T[fi2] * sg2[:, None]).astype(BF16)     # [128, S]

    # pair j holds (q-head j, q-head j+4) so lo half uses kv 0, hi half kv 1
    pair_order = [0, 4, 1, 5, 2, 6, 3, 7]
    q_cols = np.concatenate([64 * pair_order[i] + _perm
                             for i in range(H // TP)])
    o_rows = np.concatenate([np.arange(64 * pair_order[i],
                                       64 * pair_order[i] + 64)
                             for i in range(H // TP)])
    kv_perm = np.concatenate([64 * h + _perm for h in range(KV // TP)])

    in_maps = []
    for d in range(DP):
        xt = np.ascontiguousarray(x[d].T).astype(BF16)
        for t in range(TP):
            wq_s = np.ascontiguousarray(
                wq[:, HQ * t:HQ * (t + 1)][:, q_cols]).astype(BF16)
            wk_s = np.ascontiguousarray(
                wk[:, HKV * t:HKV * (t + 1)][:, kv_perm]).astype(BF16)
            wv_s = np.ascontiguousarray(
                wv[:, HKV * t:HKV * (t + 1)]).astype(BF16)
            wo_s = np.ascontiguousarray(
                wo[HQ * t:HQ * (t + 1), :][o_rows]).astype(BF16)
            in_maps.append({
                "xt": xt, "wq": wq_s, "wk": wk_s, "wv": wv_s, "wo": wo_s,
                "c1": c1, "c2": c2, "dmask": dmask,
            })

    trace = bool(os.environ.get("BASS_KERNEL_TRACE"))
    res = run_bass_kernel_spmd(nc, in_maps, core_ids=list(range(NCORES)),
                               trace=trace)
    last_exec_time_ns = res.exec_time_ns
    last_trace = res
    out = np.empty((B, S, DIM), np.float32)
    for d in range(DP):
        acc = res.results[d * TP]["out"].astype(np.float32)
        for t in range(1, TP):
            acc = acc + res.results[d * TP + t]["out"]
        out[d] = acc
    return out


# revision 41
# speedup vs baseline: 1.2098x; 1.2098x over previous
"""Trainium2 Bass kernel for GQA attention (B=2, S=2048, DIM=2048, H=32, KV=8, HD=64).

Sharding: tensor-parallel over kv heads (TP=4, 2 kv heads / 8 q heads per core)
x data-parallel over batch (DP=2).  Core c = d*4 + t.  Each core computes a
partial out = attn_out_shard @ wo_rows_shard for its batch; the host sums the
4 TP partials per batch.

All host-side work is layout-only: transpose x, permute wq/wk columns into a
RoPE-friendly even/odd layout, cast to bf16, build trig/mask pattern tiles.

Device dataflow (per core):
 - projections with x^T resident in SBUF (bf16 matmuls, k-outer accumulation)
 - RoPE via stream_shuffle + two tensor muls + add (even/odd pairs laid out
   within 32-partition quadrants)
 - attention with transposed scores (scores[sk, sq]) so no transposes are
   needed anywhere in the inner loop; exp on ScalarE with no max-subtraction
   (inputs are unit-scale; softmax is shift-invariant)
 - causal masking by block skipping + multiplicative 0/1 patterns on the
   diagonal tiles, with column-trimmed exp/mask/av on those tiles
 - softmax denominators ride as a ones-column inside the AV matmul lhsT;
   normalization = DVE reciprocal -> gpsimd partition_broadcast -> DVE mul
 - wo output projection as a final phase, outputs streamed to DRAM
"""

import os
import sys

import numpy as np

_REPO = "/opt/trn_rl_repo"
if _REPO not in sys.path:
    sys.path.insert(0, _REPO)

import ml_dtypes  # noqa: E402

BF16 = ml_dtypes.bfloat16

B, S, DIM = 2, 2048, 2048
H, KV, HD = 32, 8, 64
TP, DP = 4, 2
NCORES = TP * DP
HQ = (H // TP) * HD          # 512 q-proj cols per core
HKV = (KV // TP) * HD        # 128 kv-proj cols per core
NKVC = KV // TP              # 2 kv heads per core
NPAIR = (H // TP) // 2       # 4 q-head pairs per core
SQC = 512                    # sq chunk width
NCHUNK = S // SQC
SKT = 128                    # sk tile height
NSKT = S // SKT
KT = DIM // 128              # contraction tiles
VW = 130                     # v_sb tile: [0(32) | 1 | 0(31) | v(64) | 1 | pad]

# RoPE layout: within each head's 64 dims -> 64 partitions, quadrant q (32)
# holds pairs 16q..16q+15 as [evens(16) | odds(16)].
_perm = np.empty(64, np.int64)
_freq = np.empty(64, np.int64)
_sgn = np.empty(64, np.float32)
for _p in range(64):
    _q, _j = divmod(_p, 32)
    if _j < 16:
        _i = 16 * _q + _j
        _perm[_p] = 2 * _i
        _sgn[_p] = -1.0
    else:
        _i = 16 * _q + _j - 16
        _perm[_p] = 2 * _i + 1
        _sgn[_p] = 1.0
    _freq[_p] = _i
SHUF = list(range(16, 32)) + list(range(0, 16))

_build_cache = {}
last_exec_time_ns = None
last_trace = None


def _mask_structure(mask):
    """chunks[c] = [(t, pat_idx|None, col_trim), ...] per valid sk tile;
    patterns = list of [128, 2*SQC] float32 0/1 (duplicated for both halves
    of the mixed-half p tile)."""
    valid = mask[0, 0] == 0.0  # [sq, sk]
    chunks = []
    patterns = []
    pat_keys = {}
    for c in range(NCHUNK):
        glist = []
        for t in range(NSKT):
            sub = valid[c * SQC:(c + 1) * SQC, t * SKT:(t + 1) * SKT]
            if not sub.any():
                continue
            if sub.all():
                glist.append((t, None, 0))
                continue
            pat = np.empty((128, 2 * SQC), np.float32)
            pat[:, 0:SQC] = (sub.T - 1.0) * 240.0
            pat[:, SQC:2 * SQC] = pat[:, 0:SQC]
            key = pat.tobytes()
            if key not in pat_keys:
                pat_keys[key] = len(patterns)
                patterns.append(pat)
            # first sq column with any valid element: exp/mask/av can skip
            # columns < r (their p values are zero / never contribute)
            r = int(np.argmax(sub.any(axis=1)))
            glist.append((t, pat_keys[key], r))
        chunks.append(glist)
    return chunks, patterns


def _build(chunks, n_pat):
    import concourse.bass as bass  # noqa: F401
    import concourse.mybir as mybir
    from concourse import bacc
    from concourse.masks import make_identity
    from concourse.tile import TileContext

    F32, BF = mybir.dt.float32, mybir.dt.bfloat16
    MUL = mybir.AluOpType.mult
    ADD = mybir.AluOpType.add
    EXP = mybir.ActivationFunctionType.Exp

    nc = bacc.Bacc()
    xt_e = nc.declare_dram_parameter("xt", [DIM, S], BF, isOutput=False)
    wq_e = nc.declare_dram_parameter("wq", [DIM, HQ], BF, isOutput=False)
    wk_e = nc.declare_dram_parameter("wk", [DIM, HKV], BF, isOutput=False)
    wv_e = nc.declare_dram_parameter("wv", [DIM, HKV], BF, isOutput=False)
    wo_e = nc.declare_dram_parameter("wo", [HQ, DIM], BF, isOutput=False)
    c1_e = nc.declare_dram_parameter("c1", [128, S], BF, isOutput=False)
    c2_e = nc.declare_dram_parameter("c2", [128, S], BF, isOutput=False)
    dm_e = nc.declare_dram_parameter("dmask", [128, n_pat * 2 * SQC], BF,
                                     isOutput=False)
    out_e = nc.declare_dram_parameter("out", [S, DIM], BF, isOutput=True)

    with TileContext(nc) as tc:
        with tc.tile_pool(name="persist", bufs=1) as P:
            q_t = [P.tile([128, S], BF, tag=f"q{j}", name=f"q{j}")
                   for j in range(NPAIR)]
            k_t = P.tile([128, S], BF, tag="kt")
            v_sb = [P.tile([128, NSKT * VW], BF, tag=f"v{g}", name=f"v{g}")
                    for g in range(NKVC)]
            attn = [P.tile([128, S], BF, tag=f"a{j}", name=f"a{j}")
                    for j in range(NPAIR)]
            wo_sb = [P.tile([128, DIM], BF, tag=f"wo{j}", name=f"wo{j}")
                     for j in range(NPAIR)]
            dm_sb = P.tile([128, n_pat * 2 * SQC], BF, tag="dm")
            ident = P.tile([128, 128], BF, tag="ident")
            make_identity(nc, ident)

            for j in range(NPAIR):
                nc.gpsimd.dma_start(out=wo_sb[j],
                                    in_=wo_e[128 * j:128 * (j + 1), :])
            nc.gpsimd.dma_start(out=dm_sb, in_=dm_e[:, :])

            # v background: [0(32) | 1 | 0(31) | v | 1 | pad] per sk tile
            for g in range(NKVC):
                v3 = v_sb[g].rearrange("p (t w) -> p t w", w=VW)
                nc.vector.memset(v3[:, :, 0:32], 0.0)
                nc.vector.memset(v3[:, :, 32:33], 1.0)
                nc.vector.memset(v3[:, :, 33:64], 0.0)
                nc.vector.memset(v3[:, :, 128:129], 1.0)

            # ---------------- projections ----------------
            with (
                tc.tile_pool(name="xw", bufs=1) as XW,
                tc.tile_pool(name="ropew", bufs=2) as W,
                tc.tile_pool(name="pps", bufs=2, space="PSUM") as PPS,
            ):
                c1_sb = XW.tile([128, S], BF, tag="c1")
                c2_sb = XW.tile([128, S], BF, tag="c2")
                nc.gpsimd.dma_start(out=c1_sb, in_=c1_e[:, :])
                nc.gpsimd.dma_start(out=c2_sb, in_=c2_e[:, :])
                # x on the sync HWDGE queue, weights on the scalar queue, so
                # weight tiles land in parallel with the big x stream
                xt_sb, wq_sb, wk_sb, wv_sb = [], [], [], []
                for k in range(KT):
                    xk = XW.tile([128, S], BF, tag=f"x{k}")
                    nc.sync.dma_start(out=xk,
                                      in_=xt_e[128 * k:128 * (k + 1), :])
                    xt_sb.append(xk)
                    kk = XW.tile([128, HKV], BF, tag=f"wk{k}")
                    nc.scalar.dma_start(out=kk,
                                        in_=wk_e[128 * k:128 * (k + 1), :])
                    wk_sb.append(kk)
                    vk = XW.tile([128, HKV], BF, tag=f"wv{k}")
                    nc.scalar.dma_start(out=vk,
                                        in_=wv_e[128 * k:128 * (k + 1), :])
                    wv_sb.append(vk)
                    qk_ = XW.tile([128, HQ], BF, tag=f"wq{k}")
                    nc.scalar.dma_start(out=qk_,
                                        in_=wq_e[128 * k:128 * (k + 1), :])
                    wq_sb.append(qk_)

                def rope(dst, raw):
                    # dst = raw*c1 + shuffle(raw)*c2
                    sh = W.tile([128, S], BF, tag="sh", name="sh")
                    t1 = W.tile([128, S], BF, tag="t1", name="t1")
                    nc.vector.stream_shuffle(sh, raw, SHUF)
                    nc.vector.tensor_tensor(t1, raw, c1_sb, MUL)
                    nc.vector.tensor_tensor(sh, sh, c2_sb, MUL)
                    nc.vector.tensor_tensor(dst, t1, sh, ADD)

                def rope_project(dst, w_tiles, col0):
                    # k-outer: one ldweights feeds 4 chunk matmuls
                    raw = W.tile([128, S], BF, tag="qraw", name="raw")
                    _tags = ["ppk0", "ppk1", "ppv0", "ppv1"]
                    pss = [PPS.tile([128, SQC], F32, tag=_tags[c],
                                    name=f"pq{c}", bufs=1)
                           for c in range(NCHUNK)]
                    for k in range(KT):
                        for c in range(NCHUNK):
                            nc.tensor.matmul(
                                pss[c],
                                w_tiles[k][:, col0:col0 + 128],
                                xt_sb[k][:, SQC * c:SQC * (c + 1)],
                                start=(k == 0), stop=(k == KT - 1),
                            )
                    for c in range(NCHUNK):
                        nc.scalar.copy(raw[:, SQC * c:SQC * (c + 1)], pss[c])
                    rope(dst, raw)

                # k / v_t / q0 interleaved per k-tile in two chunk passes so
                # the PE has dense work while x/w tiles stream in
                raw_k = W.tile([128, S], BF, tag="rawk", bufs=1)
                raw_q0 = W.tile([128, S], BF, tag="rawq0", bufs=1)
                raw_q1 = W.tile([128, S], BF, tag="rawq1", bufs=1)
                vt_raw = W.tile([128, S], BF, tag="rawv", bufs=1)
                for crng in ((0, 1), (2, 3)):
                    ps_k = [PPS.tile([128, SQC], F32, tag=f"ppk{i}",
                                     name=f"ppk{i}", bufs=1)
                            for i in range(2)]
                    ps_v = [PPS.tile([128, SQC], F32, tag=f"ppv{i}",
                                     name=f"ppv{i}", bufs=1)
                            for i in range(2)]
                    ps_q = [PPS.tile([128, SQC], F32, tag=f"ppq{i}",
                                     name=f"ppq{i}", bufs=1)
                            for i in range(2)]
                    ps_q1 = [PPS.tile([128, SQC], F32, tag=f"ppr{i}",
                                      name=f"ppr{i}", bufs=1)
                             for i in range(2)]
                    for k in range(KT):
                        for ci, c in enumerate(crng):
                            xs = xt_sb[k][:, SQC * c:SQC * (c + 1)]
                            nc.tensor.matmul(
                                ps_k[ci], wk_sb[k], xs,
                                start=(k == 0), stop=(k == KT - 1))
                            nc.tensor.matmul(
                                ps_v[ci], wv_sb[k], xs,
                                start=(k == 0), stop=(k == KT - 1))
                            nc.tensor.matmul(
                                ps_q[ci], wq_sb[k][:, 0:128], xs,
                                start=(k == 0), stop=(k == KT - 1))
                            nc.tensor.matmul(
                                ps_q1[ci], wq_sb[k][:, 128:256], xs,
                                start=(k == 0), stop=(k == KT - 1))
                    for ci, c in enumerate(crng):
                        sl = slice(SQC * c, SQC * (c + 1))
                        nc.scalar.copy(raw_k[:, sl], ps_k[ci])
                        nc.scalar.copy(vt_raw[:, sl], ps_v[ci])
                        nc.scalar.copy(raw_q0[:, sl], ps_q[ci])
                        nc.scalar.copy(raw_q1[:, sl], ps_q1[ci])
                rope(k_t, raw_k)
                rope(q_t[0], raw_q0)
                rope(q_t[1], raw_q1)
                for t in range(NSKT):
                    tp = PPS.tile([128, 128], BF, tag="ppk0", bufs=1)
                    nc.tensor.transpose(tp, vt_raw[:, SKT * t:SKT * (t + 1)],
                                        ident)
                    nc.scalar.copy(
                        v_sb[0][:, VW * t + 64:VW * t + 128], tp[:, 0:64])
                    nc.scalar.copy(
                        v_sb[1][:, VW * t + 64:VW * t + 128], tp[:, 64:128])

                for j in range(2, NPAIR):
                    rope_project(q_t[j], wq_sb, 128 * j)

            # ---------------- attention ----------------
            with (
                tc.tile_pool(name="attw", bufs=2) as W,
                tc.tile_pool(name="scps", bufs=2, space="PSUM") as SCPS,
                tc.tile_pool(name="avps", bufs=1, space="PSUM") as AVPS,
            ):
                for c in range(NCHUNK):
                    glist = chunks[c]
                    for j in range(NPAIR):
                        # pair j = (q-head j -> kv 0, q-head j+4 -> kv 1)
                        # mixed-half sc tile: lo head at cols 0:512 (PE rows
                        # 0-63), hi head at cols 512:1024 (rows 64-127) --
                        # the two qk matmuls run concurrently in the array
                        pp = j % 2
                        av_lo = AVPS.tile([128, SQC], F32, tag=f"avlo{pp}",
                                          name="av_lo")
                        av_hi = AVPS.tile([128, SQC], F32, tag=f"avhi{pp}",
                                          name="av_hi")
                        for ti, (t, patk, r) in enumerate(glist):
                            first = ti == 0
                            last = ti == len(glist) - 1
                            if first:
                                r = 0  # first av matmul must cover all cols
                            sc = SCPS.tile([128, 2 * SQC], F32,
                                           tag="sc", name="sc")
                            masked = patk is not None
                            nc.tensor.matmul(
                                sc[:, r:SQC],
                                k_t[0:64, SKT * t:SKT * (t + 1)],
                                q_t[j][0:64, SQC * c + r:SQC * (c + 1)],
                                start=True, stop=not masked,
                            )
                            nc.tensor.matmul(
                                sc[:, SQC + r:2 * SQC],
                                k_t[64:128, SKT * t:SKT * (t + 1)],
                                q_t[j][64:128, SQC * c + r:SQC * (c + 1)],
                                start=True, stop=not masked,
                            )
                            if masked:
                                # additive -240 mask via PE: sc += I.T @ pat
                                base = 2 * SQC * patk
                                nc.tensor.matmul(
                                    sc[:, r:SQC], ident,
                                    dm_sb[:, base + r:base + SQC],
                                    start=False, stop=True,
                                )
                                nc.tensor.matmul(
                                    sc[:, SQC + r:2 * SQC], ident,
                                    dm_sb[:, base + SQC + r:base + 2 * SQC],
                                    start=False, stop=True,
                                )
                            p = W.tile([128, 2 * SQC], BF, tag="p", name="p",
                                       bufs=4)
                            if r:
                                sc3 = sc.rearrange(
                                    "q (h f) -> q h f", h=2)[:, :, r:SQC]
                                p3 = p.rearrange(
                                    "q (h f) -> q h f", h=2)[:, :, r:SQC]
                                nc.scalar.activation(p3, sc3, EXP,
                                                     scale=0.125)
                            else:
                                nc.scalar.activation(p, sc, EXP, scale=0.125)
                            nc.tensor.matmul(
                                av_lo[0:65, r:SQC],
                                v_sb[0][:, VW * t + 64:VW * t + 129],
                                p[:, r:SQC],
                                start=first, stop=last,
                            )
                            nc.tensor.matmul(
                                av_hi[0:128, r:SQC],
                                v_sb[1][:, VW * t:VW * t + 128],
                                p[:, SQC + r:2 * SQC],
                                start=first, stop=last,
                            )
                        rec_lo = W.tile([1, SQC], F32, tag="reclo",
                                        name="rec_lo")
                        rec_hi = W.tile([1, SQC], F32, tag="rechi",
                                        name="rec_hi")
                        rb_lo = W.tile([128, SQC], F32, tag="rblo",
                                       name="rb_lo")
                        rb_hi = W.tile([128, SQC], F32, tag="rbhi",
                                       name="rb_hi")
                        nc.vector.reciprocal(rec_lo[0:1, :], av_lo[64:65, :])
                        nc.gpsimd.partition_broadcast(rb_lo, rec_lo[0:1, :])
                        nc.vector.tensor_tensor(
                            attn[j][0:64, SQC * c:SQC * (c + 1)],
                            av_lo[0:64, :], rb_lo[0:64, :], MUL)
                        nc.vector.reciprocal(rec_hi[0:1, :], av_hi[32:33, :])
                        nc.gpsimd.partition_broadcast(rb_hi, rec_hi[0:1, :])
                        nc.vector.tensor_tensor(
                            attn[j][64:128, SQC * c:SQC * (c + 1)],
                            av_hi[64:128, :], rb_hi[64:128, :], MUL)

            # ---------------- wo (output projection) ----------------
            with (
                tc.tile_pool(name="wow", bufs=3) as W2,
                tc.tile_pool(name="ops", bufs=4, space="PSUM") as OPS,
            ):
                for s in range(S // 128):
                    o_sb = W2.tile([128, DIM], BF, tag="osb", name="o_sb")
                    for n in range(DIM // 512):
                        pso = OPS.tile([128, 512], F32, tag="pso",
                                       name="pso")
                        for j in range(NPAIR):
                            nc.tensor.matmul(
                                pso,
                                attn[j][:, 128 * s:128 * (s + 1)],
                                wo_sb[j][:, 512 * n:512 * (n + 1)],
                                start=(j == 0), stop=(j == NPAIR - 1),
                            )
                        dst = o_sb[:, 512 * n:512 * (n + 1)]
                        if n % 2 == 0:
                            nc.vector.tensor_copy(dst, pso)
                        else:
                            nc.scalar.copy(dst, pso)
                    nc.sync.dma_start(out=out_e[128 * s:128 * (s + 1), :],
                                      in_=o_sb)

    nc.finalize()
    return nc


def kernel(**inputs):
    global last_exec_time_ns, last_trace
    from concourse.bass_utils import run_bass_kernel_spmd

    x = np.asarray(inputs["x"], np.float32)
    freqs_cos = np.asarray(inputs["freqs_cos"], np.float32)
    freqs_sin = np.asarray(inputs["freqs_sin"], np.float32)
    mask = np.asarray(inputs["mask"], np.float32)
    wq = np.asarray(inputs["wq"], np.float32)
    wk = np.asarray(inputs["wk"], np.float32)
    wv = np.asarray(inputs["wv"], np.float32)
    wo = np.asarray(inputs["wo"], np.float32)

    chunks, patterns = _mask_structure(mask)
    n_pat = max(len(patterns), 1)
    if patterns:
        dmask = np.concatenate(patterns, axis=1).astype(BF16)
    else:
        dmask = np.ones((128, 2 * SQC), np.float32).astype(BF16)

    key = tuple(tuple(g) for g in chunks)
    if key not in _build_cache:
        _build_cache[key] = _build(chunks, n_pat)
    nc = _build_cache[key]

    # trig tiles in pair layout (same for both heads of a pair)
    fi2 = np.tile(_freq, 2)
    sg2 = np.tile(_sgn, 2)
    c1 = freqs_cos.T[fi2].astype(BF16)                      # [128, S]
    c2 = (freqs_sin.T[fi2] * sg2[:, None]).astype(BF16)     # [128, S]

    # pair j holds (q-head j, q-head j+4) so lo half uses kv 0, hi half kv 1
    pair_order = [0, 4, 1, 5, 2, 6, 3, 7]
    q_cols = np.concatenate([64 * pair_order[i] + _perm
                             for i in range(H // TP)])
    o_rows = np.concatenate([np.arange(64 * pair_order[i],
                                       64 * pair_order[i] + 64)
                             for i in range(H // TP)])
    kv_perm = np.concatenate([64 * h + _perm for h in range(KV // TP)])

    in_maps = []
    for d in range(DP):
        xt = np.ascontiguousarray(x[d].T).astype(BF16)
        for t in range(TP):
            wq_s = np.ascontiguousarray(
                wq[:, HQ * t:HQ * (t + 1)][:, q_cols]).astype(BF16)
            wk_s = np.ascontiguousarray(
                wk[:, HKV * t:HKV * (t + 1)][:, kv_perm]).astype(BF16)
            wv_s = np.ascontiguousarray(
                wv[:, HKV * t:HKV * (t + 1)]).astype(BF16)
            wo_s = np.ascontiguousarray(
                wo[HQ * t:HQ * (t + 1), :][o_rows]).astype(BF16)
            in_maps.append({
                "xt": xt, "wq": wq_s, "wk": wk_s, "wv": wv_s, "wo": wo_s,
                "c1": c1, "c2": c2, "dmask": dmask,
            })

    trace = bool(os.environ.get("BASS_KERNEL_TRACE"))
    res = run_bass_kernel_spmd(nc, in_maps, core_ids=list(range(NCORES)),
                               trace=trace)
    last_exec_time_ns = res.exec_time_ns
    last_trace = res
    out = np.empty((B, S, DIM), np.float32)
    for d in range(DP):
        acc = res.results[d * TP]["out"].astype(np.float32)
        for t in range(1, TP):
            acc = acc + res.results[d * TP + t]["out"]
        out[d] = acc
    return out
